# revision 5
# baseline (speedup 1.0000x reference)
"""Chord sparse-attention module kernel for 8 TRN2 NeuronCores (Bass/Tile).

Contract: kernel(**inputs) -> np.ndarray. Full (unsharded) inputs in, full
output out. Shapes hardcoded per the problem spec:
  B=2, N=4096, E=256, H=512, N_W=12 mixing layers, L=13 chord links,
  chord offsets [0, 1, 2, 4, ..., 2048].

Sharding: (batch x E-quarter) across the 8 cores — core c handles batch
c//4 and output-embedding columns [64*(c%4), 64*(c%4)+64). Each core holds
its batch's FULL sequence, so the chord gather V[:, (n+off) % N, :] is local
tile indexing (offsets that are multiples of 128) plus on-chip partition-
shifted DMA copies (offsets < 128). No inter-core communication.

Per core:
  - g-MLP and the 12 sparse-weight MLPs run on the TensorEngine in fp16
    (replicated across the 4 cores sharing a batch; hidden under the
    DVE-bound chord mix).
  - GELU + PSUM evacuation on the ScalarEngine (exact-GELU LUT).
  - The chord mix runs as fused scalar_tensor_tensor ops on the Vector
    engine (and optionally GpSimd), with fp32 state for accuracy.
  - W tables reach [row-partition, link] layout via xbar DMA transposes.
"""

import numpy as np

B, N, E, H = 2, 4096, 256, 512
N_W = 12
L = 13
OFFS = [0] + [1 << k for k in range(L - 1)]  # [0,1,2,4,...,2048]
SUB = [(l, OFFS[l]) for l in range(L) if 0 < OFFS[l] < 128]  # 7 sub-tile shifts
ALIGNED = [(l, OFFS[l] // 128) for l in range(L) if OFFS[l] >= 128]  # 5 tile-deltas
NT = N // 128  # 32 row tiles per batch
EQ = E // 4  # 64 embedding cols per core
NC = 8

# Tiles of the mix handled by GpSimd instead of the Vector engine (0..NT).
GP_TILES = 0

_cache = {}


def _build_nc():
    from concourse import bacc, tile, mybir

    f16 = mybir.dt.float16
    f32 = mybir.dt.float32
    Gelu = mybir.ActivationFunctionType.Gelu
    Copy = mybir.ActivationFunctionType.Copy
    mult = mybir.AluOpType.mult
    add = mybir.AluOpType.add

    nc = bacc.Bacc(
        "TRN2",
        target_bir_lowering=False,
        debug=False,
        enable_asserts=False,
        num_devices=NC,
    )

    # External inputs (per-core, host-prepped fp16 layouts).
    vT_d = nc.dram_tensor("vT", [2, 128, N], f16, kind="ExternalInput").ap()
    dT_d = nc.dram_tensor("dT", [2, 128, N], f16, kind="ExternalInput").ap()
    gw1_d = nc.dram_tensor("gw1", [128, 2, H], f16, kind="ExternalInput").ap()
    gw2_d = nc.dram_tensor("gw2", [128, 4, EQ], f16, kind="ExternalInput").ap()
    fw1_d = nc.dram_tensor("fw1", [128, 2 * N_W, H], f16, kind="ExternalInput").ap()
    fw2_d = nc.dram_tensor("fw2", [128, N_W, 4, 16], f16, kind="ExternalInput").ap()
    out_d = nc.dram_tensor("out", [N, EQ], f32, kind="ExternalOutput").ap()

    with tile.TileContext(nc) as tc:
        with (
            tc.tile_pool(name="const", bufs=1) as cpool,
            tc.tile_pool(name="dT", bufs=2) as dpool,
            tc.tile_pool(name="hid", bufs=6) as hpool,
            tc.tile_pool(name="wall", bufs=1) as wpool,
            tc.tile_pool(name="vc", bufs=2) as vcpool,
            tc.tile_pool(name="psA", bufs=4, space="PSUM") as psA,
        ):
            # ---- constant loads -------------------------------------------
            gw1 = cpool.tile([128, 2, H], f16)
            nc.sync.dma_start(gw1[:], gw1_d)
            gw2 = cpool.tile([128, 4, EQ], f16)
            nc.sync.dma_start(gw2[:], gw2_d)
            fw1 = cpool.tile([128, 2 * N_W, H], f16)
            nc.sync.dma_start(fw1[:], fw1_d)
            fw2 = cpool.tile([128, N_W, 4, 16], f16)
            nc.sync.dma_start(fw2[:], fw2_d)
            wall = wpool.tile([128, N_W, NT, 16], f16)

            dT = [dpool.tile([128, N], f16, tag="dT", name=f"dT{i}") for i in range(2)]
            for ke in range(2):
                nc.sync.dma_start(dT[ke][:], dT_d[ke])

            vc_cur = vcpool.tile([128, NT, EQ], f32, tag="vc")

            # ---- phase 1: g-MLP -> vc_cur ---------------------------------
            with (
                tc.tile_pool(name="vt", bufs=2) as vtpool,
                tc.tile_pool(name="vct", bufs=1) as vctpool,
                tc.tile_pool(name="psV", bufs=2, space="PSUM") as psV,
            ):
                vT = [vtpool.tile([128, N], f16, tag="vt", name=f"vT{i}") for i in range(2)]
                for ke in range(2):
                    nc.sync.dma_start(vT[ke][:], vT_d[ke])

                hidV = [hpool.tile([128, N], f16, tag="hid", name=f"hidV{i}") for i in range(4)]
                for j in range(4):
                    for half in range(2):
                        ps = []
                        for ci in range(4):
                            c = half * 4 + ci
                            p = psA.tile([128, 512], f32, tag="psA")
                            ps.append((c, p))
                        for ke in range(2):
                            for c, p in ps:
                                nc.tensor.matmul(
                                    p[:],
                                    gw1[:, ke, 128 * j : 128 * (j + 1)],
                                    vT[ke][:, 512 * c : 512 * (c + 1)],
                                    start=(ke == 0),
                                    stop=(ke == 1),
                                )
                        for c, p in ps:
                            nc.scalar.activation(
                                hidV[j][:, 512 * c : 512 * (c + 1)], p[:], Gelu
                            )

                # Vc^T = gW2q^T @ hidV  -> [EQ, N] then xbar-transpose to vc
                vcT = vctpool.tile([EQ, N], f16)
                for c in range(8):
                    pv = psV.tile([EQ, 512], f32, tag="psV")
                    for ke in range(4):
                        nc.tensor.matmul(
                            pv[:],
                            gw2[:, ke, :],
                            hidV[ke][:, 512 * c : 512 * (c + 1)],
                            start=(ke == 0),
                            stop=(ke == 3),
                        )
                    nc.scalar.activation(vcT[:, 512 * c : 512 * (c + 1)], pv[:], Copy)
                vcT16 = vcT  # fp16 [EQ, N]
                # transpose 64x128 blocks -> vc_cur[:, t, :] fp16? need f32 state:
                # xbar transpose requires 2-byte dtype; transpose to fp16 then
                # upcast per-tile on the vector engine.
                vc16 = vctpool.tile([128, NT, EQ], f16, tag="vc16")
                for t in range(NT):
                    nc.sync.dma_start_transpose(
                        vc16[:, t, :], vcT16[:, 128 * t : 128 * (t + 1)]
                    )
                nc.vector.tensor_copy(vc_cur[:], vc16[:])

            # ---- phase 2: the 12 W-MLPs (PE/ACT; runs ahead of the mix) ---
            # ---- phase 3: the 12 chord-mix layers (DVE)                 ---
            with (
                tc.tile_pool(name="sh", bufs=8) as shpool,
                tc.tile_pool(name="wt", bufs=4) as wtpool,
                tc.tile_pool(name="psW", bufs=2, space="PSUM") as psW,
            ):
                for k in range(N_W):
                    # W-MLP layer k: hid = gelu(data @ fsW1[k])
                    hidW = [hpool.tile([128, N], f16, tag="hid", name=f"hidW{k}_{i}") for i in range(4)]
                    for j in range(4):
                        for half in range(2):
                            ps = []
                            for ci in range(4):
                                c = half * 4 + ci
                                p = psA.tile([128, 512], f32, tag="psA")
                                ps.append((c, p))
                            for ke in range(2):
                                for c, p in ps:
                                    nc.tensor.matmul(
                                        p[:],
                                        fw1[:, 2 * k + ke, 128 * j : 128 * (j + 1)],
                                        dT[ke][:, 512 * c : 512 * (c + 1)],
                                        start=(ke == 0),
                                        stop=(ke == 1),
                                    )
                            for c, p in ps:
                                nc.scalar.activation(
                                    hidW[j][:, 512 * c : 512 * (c + 1)], p[:], Gelu
                                )
                    # W_T[k] = fsW2[k]^T @ hid -> [16, N] in 512-chunks, then
                    # xbar-transpose into wall[:, k, t, :].
                    for c in range(8):
                        pw = psW.tile([16, 512], f32, tag="psW")
                        for ke in range(4):
                            nc.tensor.matmul(
                                pw[:],
                                fw2[:, k, ke, :],
                                hidW[ke][:, 512 * c : 512 * (c + 1)],
                                start=(ke == 0),
                                stop=(ke == 3),
                            )
                        wt = wtpool.tile([16, 512], f16, tag="wt")
                        nc.scalar.activation(wt[:], pw[:], Copy)
                        for ti in range(4):
                            t = 4 * c + ti
                            nc.sync.dma_start_transpose(
                                wall[:, k, t, :], wt[:, 128 * ti : 128 * (ti + 1)]
                            )

                    # ---- mix layer k ----
                    # partition-shifted copies for the 7 sub-128 offsets
                    sh = {}
                    for l, s in SUB:
                        t_ = shpool.tile([128, NT, EQ], f32, tag="sh")
                        sh[s] = t_
                        nc.sync.dma_start(
                            t_[0 : 128 - s, :, :], vc_cur[s:128, :, :]
                        )
                        nc.sync.dma_start(
                            t_[128 - s : 128, 0 : NT - 1, :], vc_cur[0:s, 1:NT, :]
                        )
                        nc.sync.dma_start(
                            t_[128 - s : 128, NT - 1, :], vc_cur[0:s, 0, :]
                        )

                    acc = vcpool.tile([128, NT, EQ], f32, tag="vc")
                    for t in range(NT):
                        eng = nc.gpsimd if t >= NT - GP_TILES else nc.vector
                        # link 0 (off=0) fused with the residual: acc = vc*(W0) + vc
                        eng.scalar_tensor_tensor(
                            acc[:, t, :],
                            vc_cur[:, t, :],
                            wall[:, k, t, 0:1],
                            vc_cur[:, t, :],
                            mult,
                            add,
                        )
                        for l, d in ALIGNED:
                            eng.scalar_tensor_tensor(
                                acc[:, t, :],
                                vc_cur[:, (t + d) % NT, :],
                                wall[:, k, t, l : l + 1],
                                acc[:, t, :],
                                mult,
                                add,
                            )
                        for l, s in SUB:
                            eng.scalar_tensor_tensor(
                                acc[:, t, :],
                                sh[s][:, t, :],
                                wall[:, k, t, l : l + 1],
                                acc[:, t, :],
                                mult,
                                add,
                            )
                    vc_cur = acc

                # ---- output ----
                for t in range(NT):
                    nc.sync.dma_start(
                        out_d[128 * t : 128 * (t + 1), :], vc_cur[:, t, :]
                    )

    nc.compile()
    return nc


def _get_nc():
    if "nc" not in _cache:
        _cache["nc"] = _build_nc()
    return _cache["nc"]


def _prep_in_maps(V, data, gW1, gW2, fsW1, fsW2):
    """Host-side shard + fp16 layout prep. Returns one in_map per core."""
    f16 = np.float16
    fsW2p = np.zeros((N_W, H, 16), np.float32)
    fsW2p[:, :, :L] = fsW2
    gw1_h = np.ascontiguousarray(
        gW1.reshape(2, 128, H).transpose(1, 0, 2)
    ).astype(f16)
    fw1_h = np.ascontiguousarray(
        fsW1.reshape(N_W, 2, 128, H).transpose(2, 0, 1, 3).reshape(128, 2 * N_W, H)
    ).astype(f16)
    fw2_h = np.ascontiguousarray(
        fsW2p.reshape(N_W, 4, 128, 16).transpose(2, 0, 1, 3)
    ).astype(f16)
    in_maps = []
    for c in range(NC):
        b, q = divmod(c, 4)
        vT_h = np.ascontiguousarray(V[b].T).astype(f16).reshape(2, 128, N)
        dT_h = np.ascontiguousarray(data[b].T).astype(f16).reshape(2, 128, N)
        gw2_h = np.ascontiguousarray(
            gW2[:, EQ * q : EQ * (q + 1)].reshape(4, 128, EQ).transpose(1, 0, 2)
        ).astype(f16)
        in_maps.append(
            {
                "vT": vT_h,
                "dT": dT_h,
                "gw1": gw1_h,
                "gw2": gw2_h,
                "fw1": fw1_h,
                "fw2": fw2_h,
            }
        )
    return in_maps


def _assemble(results):
    out = np.empty((B, N, E), np.float32)
    for c in range(NC):
        b, q = divmod(c, 4)
        out[b, :, EQ * q : EQ * (q + 1)] = results[c]["out"]
    return out


def _inputs_match_contract(gb1, gb2, fsb1, fsb2, cols):
    if not (
        np.all(gb1 == 0) and np.all(gb2 == 0) and np.all(fsb1 == 0) and np.all(fsb2 == 0)
    ):
        return False
    exp_cols = (
        (np.arange(N)[:, None] + np.array(OFFS)[None, :]) % N
    ).astype(np.int64)
    return np.array_equal(np.asarray(cols).astype(np.int64), exp_cols)


def kernel(**inputs) -> np.ndarray:
    V = np.asarray(inputs["V"], np.float32)
    data = np.asarray(inputs["data"], np.float32)
    gW1 = np.asarray(inputs["gW1"], np.float32)
    gb1 = np.asarray(inputs["gb1"], np.float32)
    gW2 = np.asarray(inputs["gW2"], np.float32)
    gb2 = np.asarray(inputs["gb2"], np.float32)
    fsW1 = np.asarray(inputs["fsW1"], np.float32)
    fsb1 = np.asarray(inputs["fsb1"], np.float32)
    fsW2 = np.asarray(inputs["fsW2"], np.float32)
    fsb2 = np.asarray(inputs["fsb2"], np.float32)
    cols = inputs["cols"]

    if not _inputs_match_contract(gb1, gb2, fsb1, fsb2, cols):
        return _kernel_numpy(V, data, gW1, gb1, gW2, gb2, fsW1, fsb1, fsW2, fsb2, cols)

    from concourse import bass_utils

    nc = _get_nc()
    in_maps = _prep_in_maps(V, data, gW1, gW2, fsW1, fsW2)
    res = bass_utils.run_bass_kernel_spmd(nc, in_maps, core_ids=list(range(NC)))
    return _assemble(res.results)


# ---------------------------------------------------------------------------
# numpy fallback (only used if inputs deviate from setup_inputs() contract)
# ---------------------------------------------------------------------------


def _gelu_exact(x):
    from scipy.special import erf

    return (0.5 * x * (1.0 + erf(x / np.sqrt(2.0)))).astype(np.float32)


def _kernel_numpy(V, data, gW1, gb1, gW2, gb2, fsW1, fsb1, fsW2, fsb2, cols):
    f32 = np.float32
    Vf = V.reshape(B * N, E)
    dataf = data.reshape(B * N, E)
    hid = _gelu_exact(Vf @ gW1 + gb1)
    Vc = (hid @ gW2 + gb2).reshape(B, N, E)
    cols = np.asarray(cols)
    for k in range(N_W):
        h = _gelu_exact(dataf @ fsW1[k] + fsb1[k])
        Wk = (h @ fsW2[k] + fsb2[k]).reshape(B, N, L)
        Vg = Vc[:, cols, :]
        Vc = np.einsum("bnl,bnle->bne", Wk, Vg) + Vc
    return Vc.astype(f32)


if __name__ == "__main__":
    rng = np.random.default_rng(0)
    ins = {
        "V": rng.standard_normal((B, N, E), dtype=np.float32),
        "data": rng.standard_normal((B, N, E), dtype=np.float32),
        "gW1": rng.standard_normal((E, H), dtype=np.float32) * 0.02,
        "gb1": np.zeros((H,), np.float32),
        "gW2": rng.standard_normal((H, E), dtype=np.float32) * 0.02,
        "gb2": np.zeros((E,), np.float32),
        "fsW1": rng.standard_normal((N_W, E, H), dtype=np.float32) * 0.02,
        "fsb1": np.zeros((N_W, H), np.float32),
        "fsW2": rng.standard_normal((N_W, H, L), dtype=np.float32) * 0.02,
        "fsb2": np.zeros((N_W, L), np.float32),
        "cols": ((np.arange(N)[:, None] + np.array(OFFS)[None, :]) % N).astype(
            np.int32
        ),
    }
    out = kernel(**ins)
    ref = _kernel_numpy(**{k: np.asarray(v, np.float32) if k != "cols" else v for k, v in ins.items()})
    err = np.linalg.norm(out - ref) / np.linalg.norm(ref)
    print("shape", out.shape, "rel l2 err vs numpy:", err)
